# revision 7
# baseline (speedup 1.0000x reference)
"""TRN2 Bass kernel for nn_ConvNeXtBlock_RNN (ConvNeXt block + 2-layer tanh RNN).

Strategy: data-parallel over batch (8 rows -> 8 NeuronCores, SPMD, no
collectives). Per core: dwconv0+gelu and the (join @ ih0)-fused input
projection as bf16 PE GEMMs; the two sequential RNN scans run interleaved
(scan1 lags scan0 by one 64-step chunk) with 4-way column-tiled bf16
matvecs on the PE, f32 PSUM accumulate, f32 add + tanh, and PE mini
transposes to return the state to column form; then u1/u2 MLP + dwconv1 +
residual.  All weights are pre-transposed/fused on the host in numpy.
"""
import sys
sys.path.insert(0, '/opt/trn_rl_repo')
from contextlib import ExitStack
import numpy as np
import ml_dtypes

import concourse.bacc as bacc
import concourse.tile as tile
from concourse import mybir
from concourse.bass_utils import run_bass_kernel_spmd

F32 = mybir.dt.float32
BF16 = mybir.dt.bfloat16
AF = mybir.ActivationFunctionType

DIM = 512
IDIM = 1024
B = 8
T = 1024
CH = 64          # scan chunk (v1 GEMM granularity / scan1 lag)
SCH = 8          # seed staging subchunk
NT = IDIM // 128  # 8 i-tiles
DT = DIM // 128   # 4 d-tiles
TPAD = T + 6      # padded time axis for conv inputs

# ---- bf16 weight blob layout (cols in a [128, *] bf16 array) ----
O_WT0 = 0                      # sw(w_hh0)           8192
O_WT1 = O_WT0 + 8 * IDIM       # sw(w_hh1)           8192
O_WI1 = O_WT1 + 8 * IDIM       # sw(w_ih1)           8192
O_WU1 = O_WI1 + 8 * IDIM       # sw(w_u1)            8192
O_WU2 = O_WU1 + 8 * IDIM       # sw(w_u2) [m->d]     4096
O_CT = O_WU2 + 8 * DIM         # sw(C)   [d->j]      4096
O_DG0 = O_CT + 4 * IDIM        # conv0 diagonals     3584
O_DG1 = O_DG0 + DT * 7 * 128   # conv1 diagonals     3584
O_XB = O_DG1 + DT * 7 * 128    # padded bf16 x       4*TPAD
O_ROWS = O_XB + DT * TPAD      # row-0 bias rows: ones(128) c0(1024) c1(1024) bu1(1024) bu2(512)
O_ONES = O_ROWS
O_C0 = O_ONES + 128
O_C1 = O_C0 + IDIM
O_BU1 = O_C1 + IDIM
O_BU2 = O_BU1 + IDIM
O_IDTB = O_BU2 + DIM           # bf16 identity 128
WB_COLS = O_IDTB + 128

# ---- f32 blob layout ----
F_IDT = 0                      # identity 128
F_X = F_IDT + 128              # x f32 (residual) 4096
F_H0 = F_X + DT * T            # starter col-form 8
F_H1 = F_H0 + NT
F_CB = F_H1 + NT               # conv biases: b_dw0 (4 cols), b_dw1 (4 cols)
FB_COLS = F_CB + 2 * DT


def _sw(m, ntile):
    """[J, Kin] weight -> moving-operand layout [128, ntile*J]:
    out[p, it*J + j] = m[j, it*128 + p]."""
    j, k = m.shape
    assert k == ntile * 128
    return np.ascontiguousarray(
        m.T.reshape(ntile, 128, j).transpose(1, 0, 2).reshape(128, ntile * j))


def build(t_len=T):
    halves = [(o, min(512, t_len - o)) for o in range(0, t_len, 512)]
    nt_ch = t_len // CH
    nc = bacc.Bacc("TRN2", target_bir_lowering=False)
    wb_in = nc.declare_dram_parameter("wb", [128, WB_COLS], BF16, isOutput=False)
    fb_in = nc.declare_dram_parameter("fb", [128, FB_COLS], F32, isOutput=False)
    out_d = nc.declare_dram_parameter("out", [DIM, t_len], F32, isOutput=True)
    u0d = nc.dram_tensor("u0d", [t_len, IDIM], F32)
    v1d = nc.dram_tensor("v1d", [2, CH, IDIM], F32)
    ysd = nc.dram_tensor("ysd", [t_len // CH, 128, NT * CH], BF16)

    with tile.TileContext(nc) as tc, ExitStack() as ctx:
        cpool = ctx.enter_context(tc.tile_pool(name="const", bufs=1))
        WB = cpool.tile([128, WB_COLS], BF16)
        FB = cpool.tile([128, FB_COLS], F32)
        nc.sync.dma_start(out=WB[:, :], in_=wb_in[:, :])
        nc.sync.dma_start(out=FB[:, :], in_=fb_in[:, :])
        IDT = FB[:, F_IDT:F_IDT + 128]
        IDTB = WB[:, O_IDTB:O_IDTB + 128]

        H0 = cpool.tile([128, NT], BF16)
        H1 = cpool.tile([128, NT], BF16)
        nc.vector.tensor_copy(H0[:, :], FB[:, F_H0:F_H0 + NT])
        nc.vector.tensor_copy(H1[:, :], FB[:, F_H1:F_H1 + NT])

        # ---------------- Phase 1: conv0+gelu, u0 GEMM ----------------
        with tc.tile_pool(name="p1psum", bufs=2, space="PSUM") as p1p, \
             tc.tile_pool(name="p1work", bufs=2) as p1w, \
             tc.tile_pool(name="gsb", bufs=1) as gpool:
            # PE warmup on both DMA streams
            wrm = p1p.tile([128, 128], F32, tag="wrm")
            nc.tensor.transpose(wrm[:, :], IDT, IDT)
            nc.tensor.matmul(wrm[0:1, 0:8], lhsT=WB[:, 0:1], rhs=WB[:, 0:8],
                             start=True, stop=True)

            GSB = gpool.tile([128, DT * t_len], BF16)
            for dt in range(DT):
                for off, w in halves:
                    pc = p1p.tile([128, 512], F32, tag="convp")
                    for k in range(7):
                        nc.tensor.matmul(
                            pc[:, 0:w],
                            lhsT=WB[:, O_DG0 + (dt * 7 + k) * 128: O_DG0 + (dt * 7 + k + 1) * 128],
                            rhs=WB[:, O_XB + dt * TPAD + off + k: O_XB + dt * TPAD + off + k + w],
                            start=(k == 0), stop=(k == 6))
                    nc.scalar.activation(
                        GSB[:, dt * t_len + off: dt * t_len + off + w],
                        pc[:, 0:w], AF.Gelu, bias=FB[:, F_CB + dt:F_CB + dt + 1])
            # u0 = CT^T g + c0  -> u0d
            for tc8 in range(t_len // 128):
                for jh in range(2):
                    pu = p1p.tile([128, 512], F32, tag="u0p")
                    for dt in range(DT):
                        nc.tensor.matmul(
                            pu[:, :],
                            lhsT=GSB[:, dt * t_len + tc8 * 128: dt * t_len + tc8 * 128 + 128],
                            rhs=WB[:, O_CT + dt * IDIM + jh * 512: O_CT + dt * IDIM + jh * 512 + 512],
                            start=(dt == 0), stop=False)
                    nc.tensor.matmul(
                        pu[:, :],
                        lhsT=WB[0:1, O_ONES:O_ONES + 128],
                        rhs=WB[0:1, O_C0 + jh * 512: O_C0 + jh * 512 + 512],
                        start=False, stop=True)
                    us = p1w.tile([128, 512], F32, tag="u0s")
                    nc.scalar.activation(us[:, :], pu[:, :], AF.Identity)
                    nc.sync.dma_start(
                        out=u0d[tc8 * 128:(tc8 + 1) * 128, jh * 512:(jh + 1) * 512],
                        in_=us[:, :])

        # ---------------- Phase 2: interleaved scans ----------------
        with tc.tile_pool(name="s0psum", bufs=1, space="PSUM") as s0p, \
             tc.tile_pool(name="s1psum", bufs=1, space="PSUM") as s1p, \
             tc.tile_pool(name="tpsum", bufs=2, space="PSUM") as tpp, \
             tc.tile_pool(name="vpsum", bufs=1, space="PSUM") as vpp, \
             tc.tile_pool(name="sst", bufs=2) as sst, \
             tc.tile_pool(name="swork", bufs=2) as swk, \
             tc.tile_pool(name="stage", bufs=2) as stg:

            s0_tiles = {}
            s1_tiles = {}
            h0stage = {}
            ysstage = {}
            v1sb = {}

            def scan_step(which, t):
                if which == 0:
                    H, wtoff, seeds = H0, O_WT0, s0_tiles
                else:
                    H, wtoff, seeds = H1, O_WT1, s1_tiles
                ppool = s0p if which == 0 else s1p
                P = ppool.tile([128, 256], F32, tag=f"sp{which}")
                for g in range(4):
                    for it in range(NT):
                        nc.tensor.matmul(
                            P[32 * g:32 * g + 1, :],
                            lhsT=H[:, it:it + 1],
                            rhs=WB[:, wtoff + it * IDIM + g * 256: wtoff + it * IDIM + (g + 1) * 256],
                            start=(it == 0), stop=(it == NT - 1),
                            tile_position=(0, 32 * g))
                return P

            def scan_tail(which, t, P):
                H = H0 if which == 0 else H1
                seeds = s0_tiles if which == 0 else s1_tiles
                S = seeds[t // SCH]
                s = t % SCH
                pre = swk.tile([128, 256], F32, tag=f"pre{which}")
                nc.vector.tensor_add(pre[:, :], P[:, :], S[:, s * 256:(s + 1) * 256])
                row = swk.tile([128, 256], BF16, tag=f"row{which}")
                nc.scalar.activation(row[:, :], pre[:, :], AF.Tanh)
                PT = tpp.tile([128, 256], BF16, tag=f"pt{which}")
                for hf in range(2):
                    nc.tensor.transpose(PT[:, hf * 128:(hf + 1) * 128],
                                        row[:, hf * 128:(hf + 1) * 128], IDTB)
                for hf in range(2):
                    nc.vector.tensor_copy(H[:, hf:NT:2],
                                          PT[:, hf * 128:(hf + 1) * 128:32])
                # stage column-form state for the chunk GEMM / output
                stage = h0stage if which == 0 else ysstage
                st = stage[t // CH]
                nc.vector.tensor_copy(st[:, (t % CH)::CH], H[:, :])

            def seed_dma0(sub):
                """u0d[8 steps] -> strided rows {0,32,64,96} staging."""
                S = sst.tile([128, SCH * 256], F32, tag="s0seed")
                s0_tiles[sub] = S
                t0 = sub * SCH
                nc.sync.dma_start(
                    out=S[0:128:32, :].rearrange("q (s j) -> q s j", j=256),
                    in_=u0d[t0:t0 + SCH, :].rearrange("s (q j) -> q s j", q=4))

            def seed_dma1(sub):
                """v1d rows -> strided staging for scan1."""
                S = sst.tile([128, SCH * 256], F32, tag="s1seed")
                s1_tiles[sub] = S
                ck = (sub * SCH) // CH
                r0 = (sub * SCH) % CH
                nc.sync.dma_start(
                    out=S[0:128:32, :].rearrange("q (s j) -> q s j", j=256),
                    in_=v1d[ck % 2, r0:r0 + SCH, :].rearrange("s (q j) -> q s j", q=4))

            def v1_gemm(ck):
                st = h0stage[ck]
                v = stg.tile([CH, IDIM], F32, tag="v1sb")
                v1sb[ck] = v
                for jh in range(2):
                    pv = vpp.tile([CH, 512], F32, tag="v1p")
                    for it in range(NT):
                        nc.tensor.matmul(
                            pv[:, :],
                            lhsT=st[:, it * CH:(it + 1) * CH],
                            rhs=WB[:, O_WI1 + it * IDIM + jh * 512: O_WI1 + it * IDIM + jh * 512 + 512],
                            start=(it == 0), stop=False)
                    nc.tensor.matmul(
                        pv[:, :],
                        lhsT=WB[0:1, O_ONES:O_ONES + CH],
                        rhs=WB[0:1, O_C1 + jh * 512: O_C1 + jh * 512 + 512],
                        start=False, stop=True)
                    nc.scalar.activation(v[:, jh * 512:(jh + 1) * 512], pv[:, :],
                                         AF.Identity)
                nc.sync.dma_start(out=v1d[ck % 2, :, :], in_=v[:, :])

            def ys_out(ck):
                st = ysstage[ck]
                nc.sync.dma_start(out=ysd[ck, :, :], in_=st[:, :])

            LAG = CH
            for t in range(t_len + LAG):
                t1 = t - LAG
                if t < t_len:
                    if t % CH == 0:
                        h0stage[t // CH] = stg.tile([128, NT * CH], BF16, tag="h0st", name=f"h0st{t // CH}")
                    if t % SCH == 0:
                        seed_dma0(t // SCH)
                if 0 <= t1:
                    if t1 % CH == 0:
                        ysstage[t1 // CH] = stg.tile([128, NT * CH], BF16, tag="ysst", name=f"ysst{t1 // CH}")
                    if t1 % SCH == 0:
                        seed_dma1(t1 // SCH)
                P0 = scan_step(0, t) if t < t_len else None
                P1 = scan_step(1, t1) if 0 <= t1 else None
                if P0 is not None:
                    scan_tail(0, t, P0)
                if P1 is not None:
                    scan_tail(1, t1, P1)
                if t < t_len and t % CH == CH - 1:
                    v1_gemm(t // CH)
                if 0 <= t1 and t1 % CH == CH - 1:
                    ys_out(t1 // CH)

        # ---------------- Phase 3: u1/u2 MLP + conv1 + residual ----------------
        with tc.tile_pool(name="p3psum", bufs=2, space="PSUM") as p3p, \
             tc.tile_pool(name="p3work", bufs=2) as p3w, \
             tc.tile_pool(name="ytile", bufs=1) as ypool:
            YT = ypool.tile([128, DT * TPAD], BF16)   # conv1 input, [d, t] padded
            nc.gpsimd.memset(YT[:, :], 0.0)
            for ck in range(t_len // CH):
                ysb = p3w.tile([128, NT * CH], BF16, tag="ysb")
                nc.sync.dma_start(out=ysb[:, :], in_=ysd[ck, :, :])
                G1 = p3w.tile([CH, IDIM], BF16, tag="g1")
                for jh in range(2):
                    pg = p3p.tile([CH, 512], F32, tag="mm3")
                    for it in range(NT):
                        nc.tensor.matmul(
                            pg[:, :],
                            lhsT=ysb[:, it * CH:(it + 1) * CH],
                            rhs=WB[:, O_WU1 + it * IDIM + jh * 512: O_WU1 + it * IDIM + jh * 512 + 512],
                            start=(it == 0), stop=False)
                    nc.tensor.matmul(
                        pg[:, :],
                        lhsT=WB[0:1, O_ONES:O_ONES + CH],
                        rhs=WB[0:1, O_BU1 + jh * 512: O_BU1 + jh * 512 + 512],
                        start=False, stop=True)
                    nc.scalar.activation(G1[:, jh * 512:(jh + 1) * 512], pg[:, :],
                                         AF.Gelu)
                # transpose G1 -> [m, t] column form
                G1T = p3w.tile([128, NT * CH], BF16, tag="g1t")
                for it in range(NT):
                    ptt = p3p.tile([128, CH], BF16, tag="tp3")
                    nc.tensor.transpose(ptt[:, :], G1[0:CH, it * 128:(it + 1) * 128], IDTB[0:CH, 0:CH])
                    nc.vector.tensor_copy(G1T[:, it * CH:(it + 1) * CH], ptt[:, :])
                # u2: out [t, d] then transpose into YT [d, t]
                py = p3p.tile([CH, 512], F32, tag="mm3")
                for it in range(NT):
                    nc.tensor.matmul(
                        py[:, :],
                        lhsT=G1T[:, it * CH:(it + 1) * CH],
                        rhs=WB[:, O_WU2 + it * DIM: O_WU2 + (it + 1) * DIM],
                        start=(it == 0), stop=False)
                nc.tensor.matmul(
                    py[:, :],
                    lhsT=WB[0:1, O_ONES:O_ONES + CH],
                    rhs=WB[0:1, O_BU2:O_BU2 + DIM],
                    start=False, stop=True)
                Y2 = p3w.tile([CH, 512], BF16, tag="y2")
                nc.scalar.activation(Y2[:, :], py[:, :], AF.Gelu)
                for dt in range(DT):
                    ptt = p3p.tile([128, CH], BF16, tag="tp3")
                    nc.tensor.transpose(ptt[:, :], Y2[0:CH, dt * 128:(dt + 1) * 128], IDTB[0:CH, 0:CH])
                    nc.vector.tensor_copy(
                        YT[:, dt * TPAD + 3 + ck * CH: dt * TPAD + 3 + ck * CH + CH],
                        ptt[:, :])
            # conv1 + bias + residual
            for dt in range(DT):
                for off, w in halves:
                    pc = p3p.tile([128, 512], F32, tag="mm3")
                    for k in range(7):
                        nc.tensor.matmul(
                            pc[:, 0:w],
                            lhsT=WB[:, O_DG1 + (dt * 7 + k) * 128: O_DG1 + (dt * 7 + k + 1) * 128],
                            rhs=YT[:, dt * TPAD + off + k: dt * TPAD + off + k + w],
                            start=(k == 0), stop=(k == 6))
                    zz = p3w.tile([128, 512], F32, tag="zz")
                    nc.scalar.activation(zz[:, 0:w], pc[:, 0:w], AF.Identity,
                                         bias=FB[:, F_CB + DT + dt:F_CB + DT + dt + 1])
                    zo = p3w.tile([128, 512], F32, tag="zo")
                    nc.vector.tensor_add(
                        zo[:, 0:w], zz[:, 0:w],
                        FB[:, F_X + dt * t_len + off: F_X + dt * t_len + off + w])
                    nc.sync.dma_start(
                        out=out_d[dt * 128:(dt + 1) * 128, off:off + w],
                        in_=zo[:, 0:w])
    nc.compile()
    return nc


def _make_blobs(inputs, t_len=T):
    x = np.asarray(inputs["x"], np.float32)
    w_join = np.asarray(inputs["w_join"], np.float32)
    b_join = np.asarray(inputs["b_join"], np.float32)
    w_ih0 = np.asarray(inputs["w_ih0"], np.float32)
    b_ih0 = np.asarray(inputs["b_ih0"], np.float32)
    b_hh0 = np.asarray(inputs["b_hh0"], np.float32)
    w_hh0 = np.asarray(inputs["w_hh0"], np.float32)
    w_ih1 = np.asarray(inputs["w_ih1"], np.float32)
    b_ih1 = np.asarray(inputs["b_ih1"], np.float32)
    w_hh1 = np.asarray(inputs["w_hh1"], np.float32)
    b_hh1 = np.asarray(inputs["b_hh1"], np.float32)
    w_u1 = np.asarray(inputs["w_u1"], np.float32)
    b_u1 = np.asarray(inputs["b_u1"], np.float32)
    w_u2 = np.asarray(inputs["w_u2"], np.float32)
    b_u2 = np.asarray(inputs["b_u2"], np.float32)
    w_dw0 = np.asarray(inputs["w_dw0"], np.float32)
    b_dw0 = np.asarray(inputs["b_dw0"], np.float32)
    w_dw1 = np.asarray(inputs["w_dw1"], np.float32)
    b_dw1 = np.asarray(inputs["b_dw1"], np.float32)
    starter = np.asarray(inputs["starter"], np.float32)

    C = w_ih0 @ w_join                       # [IDIM, DIM]
    c0 = w_ih0 @ b_join + b_ih0 + b_hh0      # [IDIM]
    c1 = b_ih1 + b_hh1

    wb = np.zeros((128, WB_COLS), np.float32)
    wb[:, O_WT0:O_WT0 + 8 * IDIM] = _sw(w_hh0, NT)
    wb[:, O_WT1:O_WT1 + 8 * IDIM] = _sw(w_hh1, NT)
    wb[:, O_WI1:O_WI1 + 8 * IDIM] = _sw(w_ih1, NT)
    wb[:, O_WU1:O_WU1 + 8 * IDIM] = _sw(w_u1, NT)
    wb[:, O_WU2:O_WU2 + 8 * DIM] = _sw(w_u2, NT)
    wb[:, O_CT:O_CT + 4 * IDIM] = _sw(C, DT)
    for dt in range(DT):
        for k in range(7):
            off = O_DG0 + (dt * 7 + k) * 128
            wb[:, off:off + 128] = np.diag(w_dw0[dt * 128:(dt + 1) * 128, 0, k])
            off = O_DG1 + (dt * 7 + k) * 128
            wb[:, off:off + 128] = np.diag(w_dw1[dt * 128:(dt + 1) * 128, 0, k])
    wb[0, O_ONES:O_ONES + 128] = 1.0
    wb[0, O_C0:O_C0 + IDIM] = c0
    wb[0, O_C1:O_C1 + IDIM] = c1
    wb[0, O_BU1:O_BU1 + IDIM] = b_u1
    wb[0, O_BU2:O_BU2 + DIM] = b_u2
    wb[:, O_IDTB:O_IDTB + 128] = np.eye(128, dtype=np.float32)

    fb = np.zeros((128, FB_COLS), np.float32)
    fb[:, F_IDT:F_IDT + 128] = np.eye(128, dtype=np.float32)
    fb[:, F_H0:F_H0 + NT] = starter[0].reshape(NT, 128).T
    fb[:, F_H1:F_H1 + NT] = starter[1].reshape(NT, 128).T
    for dt in range(DT):
        fb[:, F_CB + dt] = b_dw0[dt * 128:(dt + 1) * 128]
        fb[:, F_CB + DT + dt] = b_dw1[dt * 128:(dt + 1) * 128]

    in_maps = []
    for b in range(B):
        wbb = wb.copy()
        xpad = np.zeros((128, DT * TPAD), np.float32)
        for dt in range(DT):
            xpad[:, dt * TPAD + 3: dt * TPAD + 3 + t_len] = x[b, dt * 128:(dt + 1) * 128, :]
        wbb[:, O_XB:O_XB + DT * TPAD] = xpad
        fbb = fb.copy()
        fbb[:, F_X:F_X + DT * t_len] = x[b].reshape(DT, 128, t_len).transpose(1, 0, 2).reshape(128, DT * t_len)
        in_maps.append({
            "wb": wbb.astype(ml_dtypes.bfloat16),
            "fb": fbb,
        })
    return in_maps


_CACHED = {}


def kernel(**inputs):
    x = np.asarray(inputs["x"], np.float32)
    t_len = x.shape[2]
    in_maps = _make_blobs(inputs, t_len)
    if t_len not in _CACHED:
        _CACHED[t_len] = build(t_len)
    nc = _CACHED[t_len]
    res = run_bass_kernel_spmd(nc, in_maps, list(range(B)))
    out = np.stack([res.results[b]["out"] for b in range(B)], axis=0)
    return out.astype(np.float32)


if __name__ == "__main__":
    # quick shape check
    rng = np.random.default_rng(0)
    print("module ok")
